# revision 2
# baseline (speedup 1.0000x reference)
"""Binarized conv2d kernel for Trainium2, SPMD over 8 NeuronCores.

Math (forward-value equivalent of the reference):
    real_w  = sum_k RV[k] * weights[k]          # [256,256,3,3], exact fp32 on DVE
    scale   = mean(|real_w|, axis=(1,2,3))      # per out-channel
    out     = conv2d(sign(x), sign(real_w), pad=1) * (scale * alpha)

sign(x) and sign(real_w) are {-1,0,+1} which are exact in fp8e4, so the conv
is computed with fp8 DoubleRow matmuls (exact integer accumulation in fp32
PSUM) and the per-channel scale*alpha is applied on PSUM evacuation.

Sharding: data-parallel over batch (4 images/core).  Weight prep is
tensor-parallel: each core DMAs only its 32-out-channel slice of the fp32
weights (1.18MB instead of 9.4MB), mixes+signs it, and two small AllGathers
(74KB fp8 signs + 512B f32 abs-partials per core) reassemble the full signed
weight set co-major in DRAM.  This removes ~8.2MB of per-core HBM traffic
from the critical path at kernel start (the baseline's PE sat idle ~21us
waiting for the 9.4MB weight DMA).
"""

import numpy as np
from contextlib import ExitStack

import concourse.bass as bass
import concourse.bacc as bacc
import concourse.tile as tile
from concourse import mybir
from concourse.bass_utils import run_bass_kernel_spmd
from concourse.masks import make_identity

# Problem shapes (hardcoded per contract)
B, C, H, W = 32, 256, 56, 56
K, KS = 4, 3
NCORES = 8
BL = B // NCORES            # images per core
COB = C // NCORES           # out-channels prepped per core (32)
CIQ = 4                     # ci quarters packed into partitions with co
WCOLS = (C // CIQ) * KS * KS  # 576 cols per (co, ciq) partition row
WROW = C * KS * KS          # 2304 elems per co row

PW = W + 2                  # padded width 58
PLANE = PW * PW             # 3364
PL = 3376                   # plane stride (>= 1+PLANE+1, multiple of 16)
GO = 1                      # guard offset: plane data starts at elem 1
RPC = 8                     # rows per chunk
CHUNK = RPC * PW            # 464 elems per matmul (one PSUM bank)
NCHUNK = H // RPC           # 7 chunks: first psum tile gets 3, second 4
CIH = C // 128              # 2 ci halves
COH = C // 128              # 2 co halves
TAPS = KS * KS              # 9

# rows of x covered by the first conv tile (chunks 0-2 + halo)
XA_ROWS = 3 * RPC + 1       # 25

F32 = mybir.dt.float32
FP8 = mybir.dt.float8e4

_cache = {}


def _build():
    act_dt = FP8
    nc = bacc.Bacc("TRN2", target_bir_lowering=False, debug=False,
                   num_devices=NCORES)
    x_d = nc.dram_tensor("x", [BL, C, H, W], F32, kind="ExternalInput")
    # per-core out-channel slice of the weights (fed pre-sliced from host)
    w_d = nc.dram_tensor("weights", [K, COB, C, KS, KS], F32,
                         kind="ExternalInput")
    rv_d = nc.dram_tensor("RV", [K + 1], F32, kind="ExternalInput")
    al_d = nc.dram_tensor("alpha", [C, 1, 1], F32, kind="ExternalInput")
    o_d = nc.dram_tensor("out", [BL, C, H, W], F32, kind="ExternalOutput")

    RG = [list(range(NCORES))]

    with tile.TileContext(nc) as tc, ExitStack() as ctx:
        consts = ctx.enter_context(tc.tile_pool(name="consts", bufs=1))
        wwork = ctx.enter_context(tc.tile_pool(name="wwork", bufs=1))
        xin = ctx.enter_context(tc.tile_pool(name="xin", bufs=2))
        xpads = ctx.enter_context(tc.tile_pool(name="xpads", bufs=1))
        outp = ctx.enter_context(tc.tile_pool(name="outp", bufs=2))
        dram = ctx.enter_context(tc.tile_pool(name="dram", bufs=1,
                                              space="DRAM"))

        # --- tiny constant loads on the ACT HWDGE ring ---------------------
        rv = consts.tile([128, K], F32, tag="rv")
        rv_src = bass.AP(tensor=rv_d.ap().tensor, offset=0,
                         ap=[[0, 128], [1, K]])
        nc.scalar.dma_start(out=rv, in_=rv_src)
        alpha_sb = []
        for h in range(COH):
            t = consts.tile([128, 1], F32, tag=f"alpha{h}")
            nc.scalar.dma_start(out=t,
                                in_=al_d.ap()[h * 128:(h + 1) * 128, 0, :])
            alpha_sb.append(t)

        # --- TP weight prep: slab DMAs first on the sync ring --------------
        # layout: partition p = co_local*4 + ciq ; free = 576 cols
        # (the whole per-core k-slab is contiguous in DRAM: [32co x 2304])
        wks = []
        for k in range(K):
            wk = wwork.tile([128, WCOLS], F32, tag=f"wk{k}", name=f"wk{k}")
            src = bass.AP(tensor=w_d.ap().tensor, offset=k * COB * WROW,
                          ap=[[WCOLS, 128], [1, WCOLS]])
            nc.sync.dma_start(out=wk, in_=src)
            wks.append(wk)

        # --- image-0 x load, split into row bands so conv00 starts early ---
        # band A: rows 0..24 (covers conv chunks 0-2 + halo), band B: rest
        x0t = []
        for s in range(CIH):
            xa = xin.tile([128, XA_ROWS * W], F32, tag=f"x0s{s}a", bufs=1,
                          name=f"x0s{s}a")
            nc.sync.dma_start(
                out=xa, in_=x_d.ap()[0, s * 128:(s + 1) * 128, 0:XA_ROWS]
                .rearrange("p a b -> p (a b)"))
            x0t.append(xa)
        for s in range(CIH):
            xb = xin.tile([128, (H - XA_ROWS) * W], F32, tag=f"x0s{s}b",
                          bufs=1, name=f"x0s{s}b")
            nc.sync.dma_start(
                out=xb, in_=x_d.ap()[0, s * 128:(s + 1) * 128, XA_ROWS:H]
                .rearrange("p a b -> p (a b)"))
            x0t.append(xb)

        # --- mix + sign + abs-partials on DVE/ACT as slabs land ------------
        wmix = wwork.tile([128, WCOLS], F32, tag="wmix", name="wmix")
        for k in range(K):
            nc.vector.scalar_tensor_tensor(
                wmix, wks[k], rv[:, k:k + 1], wks[k] if k == 0 else wmix,
                mybir.AluOpType.mult,
                mybir.AluOpType.bypass if k == 0 else mybir.AluOpType.add)
        wsig = wwork.tile([128, WCOLS], act_dt, tag="wsig", name="wsig")
        nc.scalar.sign(wsig, wmix)
        absp = wwork.tile([128, 1], F32, tag="absp", name="absp")
        nc.vector.tensor_reduce(absp, wmix, mybir.AxisListType.X,
                                mybir.AluOpType.add,
                                apply_absolute_value=True)

        # --- AllGather the signed fp8 slice + the f32 abs partials ---------
        wb_in = dram.tile([128, WCOLS], act_dt, tag="wb_in")
        wb_out = dram.tile([NCORES * 128, WCOLS], act_dt, tag="wb_out")
        ab_in = dram.tile([128, 1], F32, tag="ab_in")
        ab_out = dram.tile([NCORES * 128, 1], F32, tag="ab_out")
        nc.gpsimd.dma_start(out=wb_in, in_=wsig)
        nc.gpsimd.dma_start(out=ab_in, in_=absp)
        nc.gpsimd.collective_compute(
            "AllGather", mybir.AluOpType.bypass, replica_groups=RG,
            ins=[wb_in.opt()], outs=[wb_out.opt()])
        nc.gpsimd.collective_compute(
            "AllGather", mybir.AluOpType.bypass, replica_groups=RG,
            ins=[ab_in.opt()], outs=[ab_out.opt()])

        # readback: gathered tensor is exactly co-major [256, 2304] fp8
        wsgn = []
        for h in range(COH):
            t = wwork.tile([128, WROW], act_dt, tag=f"wsgn{h}",
                           name=f"wsgn{h}")
            src = bass.AP(tensor=wb_out.tensor, offset=h * 128 * WROW,
                          ap=[[WROW, 128], [1, WROW]])
            nc.gpsimd.dma_start(out=t, in_=src)
            wsgn.append(t)
        abs4 = []
        for h in range(COH):
            t = wwork.tile([128, CIQ], F32, tag=f"abs4{h}", name=f"abs4{h}")
            src = bass.AP(tensor=ab_out.tensor, offset=h * 128 * CIQ,
                          ap=[[CIQ, 128], [1, CIQ]])
            nc.gpsimd.dma_start(out=t, in_=src)
            abs4.append(t)

        # --- padded x planes: zero only the pad borders (DVE) --------------
        xpad = []
        for i in range(2):
            t = xpads.tile([128, CIH, PL], act_dt, tag=f"xpad{i}",
                           name=f"xpad{i}")
            for s in range(CIH):
                pl = t[:, s, :]
                nc.vector.memset(pl[:, 0:GO + PW + 1], 0.0)
                nc.vector.memset(
                    pl[:, GO + PW:GO + PW + H * PW].rearrange(
                        "p (r c) -> p r c", c=PW)[:, :, 0:1], 0.0)
                nc.vector.memset(
                    pl[:, GO + PW + PW - 1:GO + PW + PW - 1 + H * PW].rearrange(
                        "p (r c) -> p r c", c=PW)[:, :, 0:1], 0.0)
                nc.vector.memset(pl[:, GO + (PW - 1) * PW:PL], 0.0)
            xpad.append(t)
        ident = consts.tile([128, 128], act_dt, tag="ident")
        make_identity(nc, ident)

        wT = consts.tile([128, TAPS, COH, CIH, 128], act_dt, tag="wT")
        scale_alpha = [consts.tile([128, 1], F32, tag=f"sa{h}", name=f"sa{h}")
                       for h in range(COH)]

        # --- scale*alpha per half: sum the 4 ciq partials ------------------
        def reduce_half(h):
            absum = consts.tile([128, 1], F32, tag=f"ab{h}", name=f"ab{h}")
            nc.vector.tensor_reduce(absum, abs4[h], mybir.AxisListType.X,
                                    mybir.AluOpType.add)
            nc.vector.scalar_tensor_tensor(
                scale_alpha[h], absum, 1.0 / WROW, alpha_sb[h],
                mybir.AluOpType.mult, mybir.AluOpType.mult)

        # --- transpose one co-half's sign-weights into wT ------------------
        # Two PSUM stages per half (ps1: taps 0-5, ps0: taps 6-8); both
        # halves run before conv00, in the shadow of the x0 DMA.
        def transpose_half(h, wsg, cpsum):
            wsv = wsg.rearrange("p (ci t) -> p ci t", t=TAPS)
            stages = [("ps1", 3 * 512, 0, 6, 2), ("ps0", 4 * 512, 6, 9, 1)]
            for tag, width, ta, tb, ncopy in stages:
                tp = cpsum.tile([128, width], F32, tag=tag, bufs=1,
                                name=f"t{tag}")
                for i, (tap, ci) in enumerate(
                        [(t, c) for t in range(ta, tb) for c in range(CIH)]):
                    nc.tensor.matmul(
                        tp[:, i * 128:(i + 1) * 128],
                        wsv[:, ci * 128:(ci + 1) * 128, tap], ident,
                        start=True, stop=True)
                nt = tb - ta
                for ic in range(ncopy):
                    ca = ta + ic * nt // ncopy
                    cb = ta + (ic + 1) * nt // ncopy
                    o0 = (ca - ta) * CIH * 128
                    nc.scalar.copy(
                        wT[:, ca:cb, h, :, :],
                        tp[:, o0:o0 + (cb - ca) * CIH * 128].rearrange(
                            "p (t ci co) -> p t ci co", t=cb - ca, co=128))

        # --- load + sign one image into its padded plane -------------------
        def load(b):
            tiles = []
            for s in range(CIH):
                xs = xin.tile([128, H * W], F32, tag="xsb", name="xsb")
                nc.sync.dma_start(
                    out=xs, in_=x_d.ap()[b, s * 128:(s + 1) * 128].rearrange(
                        "p a b -> p (a b)"))
                tiles.append(xs)
            return tiles

        def sign_rows(b, s, src, r0, r1):
            xp = xpad[b % 2]
            dst = xp[:, s, GO:GO + PLANE].rearrange(
                "p (y x) -> p y x", x=PW)[:, 1 + r0:1 + r1, 1:57]
            nc.scalar.sign(dst, src.rearrange("p (y x) -> p y x", x=W))

        def sign(b, tiles):
            for s in range(CIH):
                sign_rows(b, s, tiles[s], 0, H)

        # --- conv for one (image, co-half) ---------------------------------
        # psum tiles: ps1 (3 chunks) first, then ps0 (4 chunks) — conv00's
        # first tile only needs x rows 0..24 (band A of the split x0 load).
        def conv(b, h, cpsum):
            xp = xpad[b % 2]
            osb = outp.tile([128, H * W], F32, tag="osb", name="osb")
            for tag, nch, c0 in (("ps1", 3, 0), ("ps0", 4, 3)):
                ps = cpsum.tile([128, nch * 512], F32, tag=tag, bufs=1,
                                name=tag)
                for itap in range(TAPS):
                    dy, dx = itap // KS - 1, itap % KS - 1
                    lhsT = wT[:, itap, h, :, :]
                    for j in range(nch):
                        c = c0 + j
                        off = GO + (1 + RPC * c + dy) * PW + dx
                        o = ps[:, j * 512:j * 512 + CHUNK]
                        nc.tensor.matmul(
                            o, lhsT, xp[:, :, off:off + CHUNK],
                            start=(itap == 0), stop=(itap == TAPS - 1),
                            perf_mode=mybir.MatmulPerfMode.DoubleRow)
                src = ps.rearrange("p (c e) -> p c e", e=512)[
                    :, 0:nch, 0:CHUNK].rearrange(
                    "p c (r x) -> p c r x", x=PW)[:, :, :, 1:57]
                dst = osb.rearrange("p (y x) -> p y x", x=W)[
                    :, c0 * RPC:(c0 + nch) * RPC, :].rearrange(
                    "p (c r) x -> p c r x", r=RPC)
                nc.scalar.activation(dst, src,
                                     mybir.ActivationFunctionType.Copy,
                                     bias=0.0, scale=scale_alpha[h])
                nc.scalar.dma_start(
                    out=o_d.ap()[b, h * 128:(h + 1) * 128,
                                 c0 * RPC:(c0 + nch) * RPC, :].rearrange(
                        "p a b -> p (a b)"),
                    in_=osb[:, c0 * RPC * W:(c0 + nch) * RPC * W])

        # --- schedule ------------------------------------------------------
        with tc.tile_pool(name="cpsum", bufs=1, space="PSUM") as cpsum:
            # sign image 0 per band as the split DMAs land (ACT order:
            # sign-w first — it gates the gather — then x bands)
            sign_rows(0, 0, x0t[0], 0, XA_ROWS)
            sign_rows(0, 1, x0t[1], 0, XA_ROWS)
            sign_rows(0, 0, x0t[2], XA_ROWS, H)
            sign_rows(0, 1, x0t[3], XA_ROWS, H)
            # HAM warmup: fp32 matmuls gated on the last weight slab keep the
            # PE activity window busy until the transposes arrive, so the
            # clock gate is open (2.4GHz) when the real work starts.
            for i in range(10):
                wtp = cpsum.tile([128, 512], F32, tag="tps", bufs=1,
                                 name="warm")
                nc.tensor.matmul(wtp[:, 0:464], wks[3][:, 0:128],
                                 wks[3][:, 0:464], start=True, stop=True)
            transpose_half(0, wsgn[0], cpsum)
            transpose_half(1, wsgn[1], cpsum)
            reduce_half(0)
            reduce_half(1)
            conv(0, 0, cpsum)
            xt1 = load(1)
            sign(1, xt1)
            conv(0, 1, cpsum)
            for b in range(1, BL):
                if b + 1 < BL:
                    xt = load(b + 1)   # prefetch ahead of this image's evacs
                    sign(b + 1, xt)
                conv(b, 0, cpsum)
                conv(b, 1, cpsum)
    nc.compile()
    return nc


def _get_nc():
    if "nc" not in _cache:
        _cache["nc"] = _build()
    return _cache["nc"]


def run(inputs, trace=False):
    nc = _get_nc()
    x = np.ascontiguousarray(inputs["x"], dtype=np.float32)
    w = np.ascontiguousarray(inputs["weights"], np.float32)
    in_maps = [
        {
            "x": x[c * BL:(c + 1) * BL],
            "weights": np.ascontiguousarray(w[:, c * COB:(c + 1) * COB]),
            "RV": np.ascontiguousarray(inputs["RV"], np.float32),
            "alpha": np.ascontiguousarray(inputs["alpha"], np.float32),
        }
        for c in range(NCORES)
    ]
    res = run_bass_kernel_spmd(nc, in_maps, core_ids=list(range(NCORES)),
                               trace=trace)
    out = np.concatenate([r["out"] for r in res.results], axis=0)
    return out, res


def kernel(**inputs) -> np.ndarray:
    out, _ = run(inputs, trace=False)
    return out


# revision 3
# speedup vs baseline: 1.4768x; 1.4768x over previous
"""Binarized conv2d kernel for Trainium2, SPMD over 8 NeuronCores.

Math (forward-value equivalent of the reference):
    real_w  = sum_k RV[k] * weights[k]          # [256,256,3,3], exact fp32 on DVE
    scale   = mean(|real_w|, axis=(1,2,3))      # per out-channel
    out     = conv2d(sign(x), sign(real_w), pad=1) * (scale * alpha)

sign(x) and sign(real_w) are {-1,0,+1} which are exact in fp8e4, so the conv
is computed with fp8 DoubleRow matmuls (exact integer accumulation in fp32
PSUM) and the per-channel scale*alpha is applied on PSUM evacuation.

Sharding: data-parallel over batch, 4 images per core; weights/RV/alpha
replicated. No collectives (an 8-core AllGather measures ~80us on this
fabric — host-proxied — so TP weight prep is a net loss).

Schedule: the front of the kernel is DMA-bound (~12.6MB of weights+x at the
~400GB/s per-core ceiling), so the conv passes are ordered h0 for ALL four
images first, then h1 for all four: the h1 weight half (4.7MB) is only
needed at ~60% of the kernel, off the critical path.  x images are loaded
in row bands so each image's first conv tile starts as soon as ~45% of its
pixels have landed.
"""

import numpy as np
from contextlib import ExitStack

import concourse.bass as bass
import concourse.bacc as bacc
import concourse.tile as tile
from concourse import mybir
from concourse.bass_utils import run_bass_kernel_spmd
from concourse.masks import make_identity

# Problem shapes (hardcoded per contract)
B, C, H, W = 32, 256, 56, 56
K, KS = 4, 3
NCORES = 8
BL = B // NCORES            # images per core

PW = W + 2                  # padded width 58
PLANE = PW * PW             # 3364
PL = 3376                   # plane stride (>= 1+PLANE+1, multiple of 16)
GO = 1                      # guard offset: plane data starts at elem 1
RPC = 8                     # rows per chunk
CHUNK = RPC * PW            # 464 elems per matmul (one PSUM bank)
NCHUNK = H // RPC           # 7 chunks: first psum tile gets 3, second 4
CIH = C // 128              # 2 ci halves
COH = C // 128              # 2 co halves
TAPS = KS * KS              # 9

# rows of x covered by the first conv tile (chunks 0-2 + halo)
XA_ROWS = 3 * RPC + 1       # 25

F32 = mybir.dt.float32
FP8 = mybir.dt.float8e4

_cache = {}


def _build():
    act_dt = FP8
    nc = bacc.Bacc("TRN2", target_bir_lowering=False, debug=False,
                   num_devices=NCORES)
    x_d = nc.dram_tensor("x", [BL, C, H, W], F32, kind="ExternalInput")
    w_d = nc.dram_tensor("weights", [K, C, C, KS, KS], F32, kind="ExternalInput")
    rv_d = nc.dram_tensor("RV", [K + 1], F32, kind="ExternalInput")
    al_d = nc.dram_tensor("alpha", [C, 1, 1], F32, kind="ExternalInput")
    o_d = nc.dram_tensor("out", [BL, C, H, W], F32, kind="ExternalOutput")

    with tile.TileContext(nc) as tc, ExitStack() as ctx:
        consts = ctx.enter_context(tc.tile_pool(name="consts", bufs=1))
        wstage = ctx.enter_context(tc.tile_pool(name="wstage", bufs=8))
        wwork = ctx.enter_context(tc.tile_pool(name="wwork", bufs=1))
        xin = ctx.enter_context(tc.tile_pool(name="xin", bufs=2))
        xpads = ctx.enter_context(tc.tile_pool(name="xpads", bufs=1))
        outp = ctx.enter_context(tc.tile_pool(name="outp", bufs=2))

        # --- tiny constant loads on the ACT HWDGE ring ---------------------
        rv = consts.tile([128, K], F32, tag="rv")
        rv_src = bass.AP(tensor=rv_d.ap().tensor, offset=0,
                         ap=[[0, 128], [1, K]])
        nc.scalar.dma_start(out=rv, in_=rv_src)
        alpha_sb = []
        for h in range(COH):
            t = consts.tile([128, 1], F32, tag=f"alpha{h}")
            nc.scalar.dma_start(out=t,
                                in_=al_d.ap()[h * 128:(h + 1) * 128, 0, :])
            alpha_sb.append(t)

        # --- padded x planes (one per image): zero the pad borders on DVE --
        xpad = []
        for i in range(BL):
            t = xpads.tile([128, CIH, PL], act_dt, tag=f"xpad{i}",
                           name=f"xpad{i}")
            for s in range(CIH):
                pl = t[:, s, :]
                nc.vector.memset(pl[:, 0:GO + PW + 1], 0.0)
                nc.vector.memset(
                    pl[:, GO + PW:GO + PW + H * PW].rearrange(
                        "p (r c) -> p r c", c=PW)[:, :, 0:1], 0.0)
                nc.vector.memset(
                    pl[:, GO + PW + PW - 1:GO + PW + PW - 1 + H * PW].rearrange(
                        "p (r c) -> p r c", c=PW)[:, :, 0:1], 0.0)
                nc.vector.memset(pl[:, GO + (PW - 1) * PW:PL], 0.0)
            xpad.append(t)
        ident = consts.tile([128, 128], act_dt, tag="ident")
        make_identity(nc, ident)

        wT = consts.tile([128, TAPS, COH, CIH, 128], act_dt, tag="wT")
        scale_alpha = [consts.tile([128, 1], F32, tag=f"sa{h}", name=f"sa{h}")
                       for h in range(COH)]

        # --- weight DMA for one co-half: 8 chunks (ci-half x k) ------------
        HCI = C // CIH * TAPS  # 1152 columns per ci-half
        def dma_half(h):
            wks = []
            for ci in range(CIH):
                for k in range(K):
                    wk = wstage.tile([128, HCI], F32, tag="wsb", name="wk")
                    wks.append(wk)
                    nc.sync.dma_start(
                        out=wk,
                        in_=w_d.ap()[k, h * 128:(h + 1) * 128,
                                     ci * (C // CIH):(ci + 1) * (C // CIH)]
                        .rearrange("p c a b -> p (c a b)"))
            return wks

        # mix (DVE, trailing the DMAs) + sign (ACT, per ci-half)
        def mix_half(h, wks):
            wmix = wwork.tile([128, C * TAPS], F32, tag="wmix", name="wmix")
            ws = wwork.tile([128, C * TAPS], act_dt, tag=f"wsign{h}", bufs=1,
                            name=f"wsign{h}")
            for ci in range(CIH):
                for k in range(K):
                    wk = wks[ci * K + k]
                    dst = wmix[:, ci * HCI:(ci + 1) * HCI]
                    nc.vector.scalar_tensor_tensor(
                        dst, wk, rv[:, k:k + 1], wk if k == 0 else dst,
                        mybir.AluOpType.mult,
                        mybir.AluOpType.bypass if k == 0 else
                        mybir.AluOpType.add)
                nc.scalar.sign(ws[:, ci * HCI:(ci + 1) * HCI],
                               wmix[:, ci * HCI:(ci + 1) * HCI])
            return ws, wmix

        # |real_w| row-sums + scale*alpha combine, on DVE
        def reduce_half(h, wmix):
            absum = consts.tile([128, 1], F32, tag=f"ab{h}", name=f"ab{h}")
            nc.vector.tensor_reduce(absum, wmix, mybir.AxisListType.X,
                                    mybir.AluOpType.add,
                                    apply_absolute_value=True)
            nc.vector.scalar_tensor_tensor(
                scale_alpha[h], absum, 1.0 / (C * TAPS), alpha_sb[h],
                mybir.AluOpType.mult, mybir.AluOpType.mult)

        # --- transpose one co-half's sign-weights into wT ------------------
        # Two PSUM stages; the PSUM->SBUF copies ride DVE (ACT is busy with
        # sign-x / evacuations around both call sites).
        def transpose_half(h, wsgn, cpsum):
            wsv = wsgn.rearrange("p (ci t) -> p ci t", t=TAPS)
            stages = [("ps1", 3 * 512, 0, 6), ("ps0", 4 * 512, 6, 9)]
            for tag, width, ta, tb in stages:
                tp = cpsum.tile([128, width], F32, tag=tag, bufs=1,
                                name=f"t{tag}")
                for i, (tap, ci) in enumerate(
                        [(t, c) for t in range(ta, tb) for c in range(CIH)]):
                    nc.tensor.matmul(
                        tp[:, i * 128:(i + 1) * 128],
                        wsv[:, ci * 128:(ci + 1) * 128, tap], ident,
                        start=True, stop=True)
                nc.vector.tensor_copy(
                    wT[:, ta:tb, h, :, :],
                    tp[:, 0:(tb - ta) * CIH * 128].rearrange(
                        "p (t ci co) -> p t ci co", t=tb - ta, co=128))

        # --- load + sign one image, split into row bands -------------------
        # band A: rows 0..24 (covers first conv tile + halo), band B: rest.
        def load(b):
            tiles = []
            for r0, r1, bandtag in ((0, XA_ROWS, "a"), (XA_ROWS, H, "b")):
                for s in range(CIH):
                    xs = xin.tile([128, (r1 - r0) * W], F32,
                                  tag=f"x{bandtag}{s}", name="xsb")
                    nc.sync.dma_start(
                        out=xs, in_=x_d.ap()[b, s * 128:(s + 1) * 128, r0:r1]
                        .rearrange("p a b -> p (a b)"))
                    tiles.append((s, r0, r1, xs))
            return tiles

        def sign(b, tiles):
            xp = xpad[b]
            for s, r0, r1, src in tiles:
                dst = xp[:, s, GO:GO + PLANE].rearrange(
                    "p (y x) -> p y x", x=PW)[:, 1 + r0:1 + r1, 1:57]
                nc.scalar.sign(dst, src.rearrange("p (y x) -> p y x", x=W))

        # --- conv for one (image, co-half) ---------------------------------
        # psum tiles: ps1 (3 chunks) first — it only needs x band A.
        def conv(b, h, cpsum):
            xp = xpad[b]
            osb = outp.tile([128, H * W], F32, tag="osb", name="osb")
            for tag, nch, c0 in (("ps1", 3, 0), ("ps0", 4, 3)):
                ps = cpsum.tile([128, nch * 512], F32, tag=tag, bufs=1,
                                name=tag)
                for itap in range(TAPS):
                    dy, dx = itap // KS - 1, itap % KS - 1
                    lhsT = wT[:, itap, h, :, :]
                    for j in range(nch):
                        c = c0 + j
                        off = GO + (1 + RPC * c + dy) * PW + dx
                        o = ps[:, j * 512:j * 512 + CHUNK]
                        nc.tensor.matmul(
                            o, lhsT, xp[:, :, off:off + CHUNK],
                            start=(itap == 0), stop=(itap == TAPS - 1),
                            perf_mode=mybir.MatmulPerfMode.DoubleRow)
                src = ps.rearrange("p (c e) -> p c e", e=512)[
                    :, 0:nch, 0:CHUNK].rearrange(
                    "p c (r x) -> p c r x", x=PW)[:, :, :, 1:57]
                dst = osb.rearrange("p (y x) -> p y x", x=W)[
                    :, c0 * RPC:(c0 + nch) * RPC, :].rearrange(
                    "p (c r) x -> p c r x", r=RPC)
                nc.scalar.activation(dst, src,
                                     mybir.ActivationFunctionType.Copy,
                                     bias=0.0, scale=scale_alpha[h])
                nc.scalar.dma_start(
                    out=o_d.ap()[b, h * 128:(h + 1) * 128,
                                 c0 * RPC:(c0 + nch) * RPC, :].rearrange(
                        "p a b -> p (a b)"),
                    in_=osb[:, c0 * RPC * W:(c0 + nch) * RPC * W])

        # --- schedule ------------------------------------------------------
        # sync-ring DMA order: w-h0, x0..x3, w-h1 (h1 weights are only
        # needed at ~60% of the kernel).
        with tc.tile_pool(name="cpsum", bufs=1, space="PSUM") as cpsum:
            wks0 = dma_half(0)
            xt = [load(0)]
            sign(0, xt[0])
            ws0, wm0 = mix_half(0, wks0)
            # HAM warmup: fp32 matmuls gated on a mid-stream weight chunk
            # bridge the PE activity window until the transposes arrive, so
            # the clock gate is open (2.4GHz) when the convs start.
            for i in range(8):
                wtp = cpsum.tile([128, 512], F32, tag="tps", bufs=1,
                                 name="warm")
                nc.tensor.matmul(wtp[:, 0:464], wks0[3][:, 0:128],
                                 wks0[3][:, 0:464], start=True, stop=True)
            transpose_half(0, ws0, cpsum)
            reduce_half(0, wm0)
            # h0 pass over all images, with x prefetch one image ahead
            for b in range(BL):
                if b + 1 < BL:
                    xt.append(load(b + 1))
                    sign(b + 1, xt[b + 1])
                else:
                    wks1 = dma_half(1)
                    ws1, wm1 = mix_half(1, wks1)
                    reduce_half(1, wm1)
                conv(b, 0, cpsum)
            # h1 pass
            transpose_half(1, ws1, cpsum)
            for b in range(BL):
                conv(b, 1, cpsum)
    nc.compile()
    return nc


def _get_nc():
    if "nc" not in _cache:
        _cache["nc"] = _build()
    return _cache["nc"]


def run(inputs, trace=False):
    nc = _get_nc()
    x = np.ascontiguousarray(inputs["x"], dtype=np.float32)
    in_maps = [
        {
            "x": x[c * BL:(c + 1) * BL],
            "weights": np.ascontiguousarray(inputs["weights"], np.float32),
            "RV": np.ascontiguousarray(inputs["RV"], np.float32),
            "alpha": np.ascontiguousarray(inputs["alpha"], np.float32),
        }
        for c in range(NCORES)
    ]
    res = run_bass_kernel_spmd(nc, in_maps, core_ids=list(range(NCORES)),
                               trace=trace)
    out = np.concatenate([r["out"] for r in res.results], axis=0)
    return out, res


def kernel(**inputs) -> np.ndarray:
    out, _ = run(inputs, trace=False)
    return out
